# revision 21
# baseline (speedup 1.0000x reference)
"""NT-Xent contrastive loss on 8 Trainium2 NeuronCores (symmetric fp8 v3).

Math: z = l2-normalize rows of concat(emb_i, emb_j) -> [8192, 512].
sim = (z @ z.T)/T, T=0.5.  denom_r = sum_j exp(sim_rj) - e^2.
loss = (sum_r ln(denom_r) - 4*sum_k cos_k) / 8192.

sim is symmetric: only the upper triangle of the 16x16 grid of 512-row
strips is computed.  exp(sim) block (r,c) contributes its row sums to
denom[strip r] and its column sums to denom[strip c].  Round-robin
pairing makes the program uniform across cores: core k receives repsT
with columns rotated left by 512k, owns LOCAL row strips 0 and 8, and
computes strip 0 x local cstrips 0..8 plus strip 8 x local cstrips
8..15.  Over k=0..7 this covers each unordered strip pair exactly once.
Per-core partial row/col sums of exp go back to the host, which
assembles denom, takes float64 log, and forms the loss.

Device pipeline per core:
  - repsT arrives bf16 (host cast); per 1024-col group: DVE squares ->
    fp8, ones-weights DoubleRow matmul -> column sums-of-squares
    (PSUM, replicated over partitions), ACT ln then exp(-.5*ln+ln 16)
    -> B = 16/||col|| (bf16), DVE scale-mul -> z tiles (fp8, x16 to
    stay clear of fp8 denormals; exp scale compensates by 1/256)
  - mains: fp8 DoubleRow matmuls (K=512 as 2 passes of 2x128), PSUM
    [128,1024] groups, ACT exp(scale=2/256) -> es bf16 + accum_out row
    sums; colsum: ones-bf16 matmul chains over the 4 row tiles of each
    off-diag 512-block -> PSUM -> DMA one partition row to DRAM
  - positive pairs: separate bf16 row-major path (DVE fused
    multiply-reduce), cos_k per pair -> DRAM
  - ACT functions (Exp/Ln/Copy) pinned to one table set -> single
    ACT_TABLE_LOAD
"""

import functools
import math
import os

import numpy as np
import ml_dtypes

import concourse.bacc as bacc
import concourse.bass as bass
import concourse.tile as tile
from concourse import mybir
from concourse.bass_utils import run_bass_kernel_spmd
from concourse.hw_specs import get_activation_tables as _orig_gat

F32 = mybir.dt.float32
BF16 = mybir.dt.bfloat16
FP8 = mybir.dt.float8e4
AF = mybir.ActivationFunctionType
ALU = mybir.AluOpType
DR = mybir.MatmulPerfMode.DoubleRow

N_CORES = 8
N = 4096              # rows per input
D = 512               # embedding dim
M = 2 * N             # 8192 rows of sim
NSTRIP = 16           # 512-row strips
SW = 512              # strip width
GW = 1024             # column group width (PSUM group size)
POS_PER_CORE = N // N_CORES       # 512
E2 = float(math.exp(2.0))
INV_T = 2.0           # 1 / temperature
ZSCALE = 16.0         # fp8 z pre-scale (avoids fp8 denormals)
EXP_SCALE = INV_T / (ZSCALE * ZSCALE)

_ONE_SET = "natural_log_exp_and_others"

# mains subgroups, uniform for every core (local indices):
#   (strip_sel, group, col_off, width); strip A = local strip 0
#   (lhsT = group 0 cols [0,512)), strip B = local strip 8 (lhsT =
#   group 4 cols [0,512)).
SUBS = (
    (0, 0, 0, 1024),
    (0, 1, 0, 1024),
    (0, 2, 0, 1024),
    (0, 3, 0, 1024),
    (0, 4, 0, 512),
    (1, 4, 0, 1024),
    (1, 5, 0, 1024),
    (1, 6, 0, 1024),
    (1, 7, 0, 1024),
)
_DIAG_CSUB = {0: 0, 1: 8}  # strip_sel -> local diag cstrip


def _sub_csubs(sub):
    """Local 512-col strips covered by a mains subgroup, with the
    diagonal one excluded (those need no colsum)."""
    s, g, off, w = sub
    c0 = (g * GW + off) // SW
    return [c for c in range(c0, c0 + w // SW) if c != _DIAG_CSUB[s]]


COLSUM_LIST = [(si, c) for si, sub in enumerate(SUBS) for c in _sub_csubs(sub)]
assert len(COLSUM_LIST) == 15


@functools.cache
def _patched_gat(arch):
    """Pin every ACT function this kernel uses to one table set so the
    table-load chooser emits a single ACT_TABLE_LOAD."""
    t = dict(_orig_gat(arch))
    if _ONE_SET not in t:
        return t
    mine = {AF.Exp, AF.Ln, AF.Square, AF.Copy, AF.Identity}
    return {
        name: (s if name == _ONE_SET else (set(s) - mine))
        for name, s in t.items()
    }


USE_BF16 = os.environ.get("K_BF16", "") != ""         # bf16 instead of fp8
USE_DR = os.environ.get("K_NO_DR", "") == "" and not USE_BF16
USE_COLSUMS = os.environ.get("K_NO_COLSUMS", "") == ""
USE_POS = os.environ.get("K_NO_POS", "") == ""
ZDT = BF16 if USE_BF16 else FP8
SQ_GP8 = int(os.environ.get("K_SQ_GP8", "5"))   # of every 8 squares, this many on gpsimd
POS_GP = os.environ.get("K_POS_DVE", "") == ""  # pos products on gpsimd


def build_program():
    bacc.get_activation_tables = _patched_gat

    nc = bacc.Bacc(
        "TRN2",
        target_bir_lowering=False,
        debug=False,
        num_devices=N_CORES,
    )

    repsT = nc.dram_tensor("repsT", [D, M], BF16, kind="ExternalInput")
    pi = nc.dram_tensor("pi", [POS_PER_CORE, D], BF16, kind="ExternalInput")
    pj = nc.dram_tensor("pj", [POS_PER_CORE, D], BF16, kind="ExternalInput")
    out_row = nc.dram_tensor("out_row", [128, 64], F32, kind="ExternalOutput")
    out_col = nc.dram_tensor("out_col", [1, 15 * SW], F32, kind="ExternalOutput")
    out_pos = nc.dram_tensor("out_pos", [128, 4], F32, kind="ExternalOutput")

    with tile.TileContext(nc) as tc:
        import contextlib

        with contextlib.ExitStack() as ctx:
            const = ctx.enter_context(tc.tile_pool(name="const", bufs=1))
            big = ctx.enter_context(tc.tile_pool(name="big", bufs=1))
            stage = ctx.enter_context(tc.tile_pool(name="stage", bufs=8))
            sqp = ctx.enter_context(tc.tile_pool(name="sqp", bufs=4))
            lnpool = ctx.enter_context(tc.tile_pool(name="lnpool", bufs=2))
            bpool = ctx.enter_context(tc.tile_pool(name="bpool", bufs=3))
            esp = ctx.enter_context(tc.tile_pool(name="esp", bufs=8))
            posp = ctx.enter_context(tc.tile_pool(name="posp", bufs=4))
            sink = ctx.enter_context(tc.tile_pool(name="sink", bufs=2))

            ones_bf = const.tile([128, 128], BF16)
            nc.vector.memset(ones_bf[:], 1.0)
            ones_dr = const.tile([128, 2, 128], ZDT)
            nc.vector.memset(ones_dr[:], 1.0)
            ln_zs = const.tile([128, 1], F32)
            nc.vector.memset(ln_zs[:], float(math.log(ZSCALE)))

            # resident z tiles: per 1024-group, two chunk-pair tiles
            # [128, 2, GW] fp8 (pair A = K rows 0..255, pair B = 256..511)
            zq = [
                [big.tile([128, 2, GW], ZDT, tag=f"zq{g}{p}",
                          name=f"zq{g}{p}") for p in range(2)]
                for g in range(8)
            ]
            dacc = big.tile([128, 64], F32, tag="dacc")
            nc.vector.memset(dacc[:], 0.0)
            colrow = big.tile([1, 15 * SW], F32, tag="colrow")
            pos_ssi = big.tile([128, 4], F32, tag="pos_ssi")
            pos_ssj = big.tile([128, 4], F32, tag="pos_ssj")
            pos_dot = big.tile([128, 4], F32, tag="pos_dot")

            pp = ctx.enter_context(
                tc.tile_pool(name="pp", bufs=2, space="PSUM")
            )
            pc = ctx.enter_context(
                tc.tile_pool(name="pc", bufs=4, space="PSUM")
            )

            def prep(g):
                pt = pp.tile([128, GW], F32, tag="pp", name=f"pt{g}")
                sq = [sqp.tile([128, 2, GW], ZDT, tag="sqp",
                               name=f"sq{g}{p}") for p in range(2)]
                sts = []
                for d in range(4):
                    st = stage.tile([128, GW], BF16, tag="stage",
                                    name=f"st{g}{d}")
                    nc.sync.dma_start(
                        st[:], repsT[bass.ts(d, 128), bass.ts(g, GW)]
                    )
                    sts.append(st)
                    eng = nc.gpsimd if (g * 4 + d) % 8 < SQ_GP8 else nc.vector
                    eng.tensor_mul(sq[d // 2][:, d % 2, :], st[:], st[:])
                if USE_DR:
                    for p in range(2):
                        for jj in range(2):
                            nc.tensor.matmul(
                                pt[:, bass.ts(jj, 512)],
                                ones_dr[:],
                                sq[p][:, :, bass.ts(jj, 512)],
                                start=(p == 0), stop=(p == 1),
                                perf_mode=DR,
                            )
                else:
                    for p in range(2):
                        for q in range(2):
                            for jj in range(2):
                                nc.tensor.matmul(
                                    pt[:, bass.ts(jj, 512)],
                                    ones_dr[:, 0, :],
                                    sq[p][:, q, bass.ts(jj, 512)],
                                    start=(p == 0 and q == 0),
                                    stop=(p == 1 and q == 1),
                                )
                lt = lnpool.tile([128, GW], BF16, tag="lnpool", name=f"lt{g}")
                nc.scalar.activation(lt[:], pt[:], AF.Ln)
                bt = bpool.tile([128, GW], BF16, tag="bpool", name=f"B{g}")
                nc.scalar.activation(bt[:], lt[:], AF.Exp, scale=-0.5,
                                     bias=ln_zs[:])
                for d in range(4):
                    nc.vector.tensor_mul(
                        zq[g][d // 2][:, d % 2, :], sts[d][:], bt[:]
                    )

            es_tiles = {}

            def mains(si):
                s, g, off, w = SUBS[si]
                lg = 0 if s == 0 else 4  # lhsT group (local cstrip 0 / 8)
                for i in range(4):
                    pm = pp.tile([128, GW], F32, tag="pp",
                                 name=f"pm{si}_{i}")
                    if USE_DR:
                        for p in range(2):
                            for jj in range(w // 512):
                                nc.tensor.matmul(
                                    pm[:, bass.ts(jj, 512)],
                                    zq[lg][p][:, :, bass.ts(i, 128)],
                                    zq[g][p][:, :, off + jj * 512:
                                             off + jj * 512 + 512],
                                    start=(p == 0), stop=(p == 1),
                                    perf_mode=DR,
                                )
                    else:
                        for p in range(2):
                            for q in range(2):
                                for jj in range(w // 512):
                                    nc.tensor.matmul(
                                        pm[:, bass.ts(jj, 512)],
                                        zq[lg][p][:, q, bass.ts(i, 128)],
                                        zq[g][p][:, q, off + jj * 512:
                                                 off + jj * 512 + 512],
                                        start=(p == 0 and q == 0),
                                        stop=(p == 1 and q == 1),
                                    )
                    es = esp.tile([128, GW], BF16, tag="esp",
                                  name=f"es{si}_{i}")
                    slot = s * 32 + i * 8 + g
                    nc.scalar.activation(
                        es[:, 0:w], pm[:, 0:w], AF.Exp, scale=EXP_SCALE,
                        accum_out=dacc[:, slot:slot + 1],
                    )
                    es_tiles[(si, i)] = es

            col_j = 0

            def colsums(si):
                nonlocal col_j
                if not USE_COLSUMS:
                    return
                s, g, off, w = SUBS[si]
                c0 = (g * GW + off) // SW
                for c in _sub_csubs(SUBS[si]):
                    eoff = (c - c0) * SW
                    pcT = pc.tile([128, SW], F32, tag="pc",
                                  name=f"pc{si}_{c}")
                    for i in range(4):
                        nc.tensor.matmul(
                            pcT[:],
                            ones_bf[:],
                            es_tiles[(si, i)][:, eoff:eoff + SW],
                            start=(i == 0), stop=(i == 3),
                        )
                    nc.vector.tensor_copy(
                        colrow[0:1, col_j * SW:(col_j + 1) * SW], pcT[0:1, :]
                    )
                    col_j += 1

            def pos_products():
                for t in range(4):
                    pit = posp.tile([128, D], BF16, tag="posp")
                    nc.sync.dma_start(pit[:], pi[bass.ts(t, 128), :])
                    pjt = posp.tile([128, D], BF16, tag="posp")
                    nc.sync.dma_start(pjt[:], pj[bass.ts(t, 128), :])
                    for src0, src1, acc in (
                        (pit, pit, pos_ssi),
                        (pjt, pjt, pos_ssj),
                        (pit, pjt, pos_dot),
                    ):
                        snk = sink.tile([128, D], BF16, tag="sink")
                        if POS_GP:
                            nc.vector.scalar_tensor_tensor(
                                snk[:], src0[:], 1.0, src1[:],
                                ALU.mult, ALU.mult, accum_out=acc[:, t:t + 1],
                            )
                        else:
                            nc.vector.tensor_mul(snk[:], src0[:], src1[:])
                            nc.vector.tensor_reduce(
                                acc[:, t:t + 1], snk[:],
                                axis=mybir.AxisListType.X, op=ALU.add,
                            )

            def pos_finish():
                lssi = big.tile([128, 4], F32, tag="lssi")
                lssj = big.tile([128, 4], F32, tag="lssj")
                nc.scalar.activation(lssi[:], pos_ssi[:], AF.Ln)
                nc.scalar.activation(lssj[:], pos_ssj[:], AF.Ln)
                lsum = big.tile([128, 4], F32, tag="lsum")
                nc.vector.tensor_add(lsum[:], lssi[:], lssj[:])
                rinv_ij = big.tile([128, 4], F32, tag="rinv_ij")
                nc.scalar.activation(rinv_ij[:], lsum[:], AF.Exp, scale=-0.5)
                posk = big.tile([128, 4], F32, tag="posk")
                nc.vector.tensor_mul(posk[:], pos_dot[:], rinv_ij[:])
                nc.sync.dma_start(out_pos[:], posk[:])

            # ------- software-pipelined schedule -----------------------
            prep(0)
            prep(1)
            if USE_POS:
                pos_products()
            mains(0)
            prep(2)
            mains(1)
            colsums(0)
            prep(3)
            mains(2)
            colsums(1)
            prep(4)
            mains(3)
            colsums(2)
            prep(5)
            mains(4)
            colsums(3)
            prep(6)
            mains(5)
            colsums(4)
            prep(7)
            mains(6)
            colsums(5)
            mains(7)
            colsums(6)
            mains(8)
            colsums(7)
            colsums(8)
            if USE_POS:
                pos_finish()
            else:
                posk = big.tile([128, 4], F32, tag="posk")
                nc.vector.memset(posk[:], 0.0)
                nc.sync.dma_start(out_pos[:], posk[:])
            nc.sync.dma_start(out_row[:], dacc[:])
            if USE_COLSUMS:
                nc.sync.dma_start(out_col[:], colrow[:])
            else:
                nc.vector.memset(colrow[:], 0.0)
                nc.sync.dma_start(out_col[:], colrow[:])

    nc.compile()
    return nc


_NC_CACHE = None


def _get_program():
    global _NC_CACHE
    if _NC_CACHE is None:
        _NC_CACHE = build_program()
    return _NC_CACHE


def make_in_maps(emb_i: np.ndarray, emb_j: np.ndarray):
    emb_i = np.asarray(emb_i, dtype=np.float32)
    emb_j = np.asarray(emb_j, dtype=np.float32)
    reps = np.concatenate([emb_i, emb_j], axis=0)          # [8192, 512]
    repsT = np.ascontiguousarray(reps.T).astype(ml_dtypes.bfloat16)
    in_maps = []
    for c in range(N_CORES):
        in_maps.append(
            {
                "repsT": np.ascontiguousarray(
                    np.roll(repsT, -SW * c, axis=1)
                ),
                "pi": emb_i[c * POS_PER_CORE:(c + 1) * POS_PER_CORE].astype(
                    ml_dtypes.bfloat16
                ),
                "pj": emb_j[c * POS_PER_CORE:(c + 1) * POS_PER_CORE].astype(
                    ml_dtypes.bfloat16
                ),
            }
        )
    return in_maps


def combine_outputs(results):
    """Assemble denom from per-core partial row/col sums, then the loss."""
    rs = np.zeros(M, dtype=np.float64)
    cos_sum = 0.0
    for k, r in enumerate(results):
        dacc = np.asarray(r["out_row"], dtype=np.float64)    # [128, 64]
        for s, rstrip in enumerate((k, k + 8)):
            base = rstrip * SW
            for i in range(4):
                sl = dacc[:, s * 32 + i * 8:s * 32 + i * 8 + 8].sum(axis=1)
                rs[base + 128 * i: base + 128 * i + 128] += sl
        ocol = np.asarray(r["out_col"], dtype=np.float64).reshape(15, SW)
        for j, (si, c_loc) in enumerate(COLSUM_LIST):
            c_glob = (k + c_loc) % NSTRIP
            rs[c_glob * SW:(c_glob + 1) * SW] += ocol[j]
        cos_sum += float(np.asarray(r["out_pos"], dtype=np.float64).sum())
    denom = rs - E2
    loss = (np.log(denom).sum() - 2.0 * INV_T * cos_sum) / float(M)
    return np.float32(loss)


def kernel(emb_i: np.ndarray, emb_j: np.ndarray) -> np.ndarray:
    nc = _get_program()
    in_maps = make_in_maps(emb_i, emb_j)
    res = run_bass_kernel_spmd(nc, in_maps, list(range(N_CORES)))
    return combine_outputs(res.results)


# revision 26
# speedup vs baseline: 1.0480x; 1.0480x over previous
"""NT-Xent contrastive loss on 8 Trainium2 NeuronCores (symmetric fp8 v3).

Math: z = l2-normalize rows of concat(emb_i, emb_j) -> [8192, 512].
sim = (z @ z.T)/T, T=0.5.  denom_r = sum_j exp(sim_rj) - e^2.
loss = (sum_r ln(denom_r) - 4*sum_k cos_k) / 8192.

sim is symmetric: only the upper triangle of the 16x16 grid of 512-row
strips is computed.  exp(sim) block (r,c) contributes its row sums to
denom[strip r] and its column sums to denom[strip c].  Round-robin
pairing makes the program uniform across cores: core k receives repsT
with columns rotated left by 512k, owns LOCAL row strips 0 and 8, and
computes strip 0 x local cstrips 0..8 plus strip 8 x local cstrips
8..15.  Over k=0..7 this covers each unordered strip pair exactly once.
Per-core partial row/col sums of exp go back to the host, which
assembles denom, takes float64 log, and forms the loss.

Device pipeline per core:
  - repsT arrives bf16 (host cast); per 1024-col group: DVE squares ->
    fp8, ones-weights DoubleRow matmul -> column sums-of-squares
    (PSUM, replicated over partitions), ACT ln then exp(-.5*ln+ln 16)
    -> B = 16/||col|| (bf16), DVE scale-mul -> z tiles (fp8, x16 to
    stay clear of fp8 denormals; exp scale compensates by 1/256)
  - mains: fp8 DoubleRow matmuls (K=512 as 2 passes of 2x128), PSUM
    [128,1024] groups, ACT exp(scale=2/256) -> es bf16 + accum_out row
    sums; colsum: ones-bf16 matmul chains over the 4 row tiles of each
    off-diag 512-block -> PSUM -> DMA one partition row to DRAM
  - positive pairs: separate bf16 row-major path (DVE fused
    multiply-reduce), cos_k per pair -> DRAM
  - ACT functions (Exp/Ln/Copy) pinned to one table set -> single
    ACT_TABLE_LOAD
"""

import functools
import math
import os

import numpy as np
import ml_dtypes

import concourse.bacc as bacc
import concourse.bass as bass
import concourse.tile as tile
from concourse import mybir
from concourse.bass_utils import run_bass_kernel_spmd
from concourse.hw_specs import get_activation_tables as _orig_gat

F32 = mybir.dt.float32
BF16 = mybir.dt.bfloat16
FP8 = mybir.dt.float8e4
AF = mybir.ActivationFunctionType
ALU = mybir.AluOpType
DR = mybir.MatmulPerfMode.DoubleRow

N_CORES = 8
N = 4096              # rows per input
D = 512               # embedding dim
M = 2 * N             # 8192 rows of sim
NSTRIP = 16           # 512-row strips
SW = 512              # strip width
GW = 1024             # column group width (PSUM group size)
POS_PER_CORE = N // N_CORES       # 512
E2 = float(math.exp(2.0))
INV_T = 2.0           # 1 / temperature
ZSCALE = 16.0         # fp8 z pre-scale (avoids fp8 denormals)
EXP_SCALE = INV_T / (ZSCALE * ZSCALE)

_ONE_SET = "natural_log_exp_and_others"

# mains subgroups, uniform for every core (local indices):
#   (strip_sel, group, col_off, width); strip A = local strip 0
#   (lhsT = group 0 cols [0,512)), strip B = local strip 8 (lhsT =
#   group 4 cols [0,512)).
SUBS = (
    (0, 0, 0, 1024),
    (0, 1, 0, 1024),
    (0, 2, 0, 1024),
    (0, 3, 0, 1024),
    (0, 4, 0, 512),
    (1, 4, 0, 1024),
    (1, 5, 0, 1024),
    (1, 6, 0, 1024),
    (1, 7, 0, 1024),
)
_DIAG_CSUB = {0: 0, 1: 8}  # strip_sel -> local diag cstrip


def _sub_csubs(sub):
    """Local 512-col strips covered by a mains subgroup, with the
    diagonal one excluded (those need no colsum)."""
    s, g, off, w = sub
    c0 = (g * GW + off) // SW
    return [c for c in range(c0, c0 + w // SW) if c != _DIAG_CSUB[s]]


COLSUM_LIST = [(si, c) for si, sub in enumerate(SUBS) for c in _sub_csubs(sub)]
assert len(COLSUM_LIST) == 15


@functools.cache
def _patched_gat(arch):
    """Pin every ACT function this kernel uses to one table set so the
    table-load chooser emits a single ACT_TABLE_LOAD."""
    t = dict(_orig_gat(arch))
    if _ONE_SET not in t:
        return t
    mine = {AF.Exp, AF.Ln, AF.Square, AF.Copy, AF.Identity}
    return {
        name: (s if name == _ONE_SET else (set(s) - mine))
        for name, s in t.items()
    }


USE_BF16 = os.environ.get("K_BF16", "") != ""         # bf16 instead of fp8
USE_DR = os.environ.get("K_NO_DR", "") == "" and not USE_BF16
USE_COLSUMS = os.environ.get("K_NO_COLSUMS", "") == ""
USE_POS = os.environ.get("K_NO_POS", "") == ""
ZDT = BF16 if USE_BF16 else FP8
SQ_GP8 = int(os.environ.get("K_SQ_GP8", "5"))   # of every 8 squares, this many on gpsimd
POS_GP = os.environ.get("K_POS_DVE", "") == ""  # pos products on gpsimd


def build_program():
    bacc.get_activation_tables = _patched_gat

    nc = bacc.Bacc(
        "TRN2",
        target_bir_lowering=False,
        debug=False,
        num_devices=N_CORES,
    )

    repsT = nc.dram_tensor("repsT", [D, M], BF16, kind="ExternalInput")
    out_row = nc.dram_tensor("out_row", [128, 64], F32, kind="ExternalOutput")
    out_col = nc.dram_tensor("out_col", [1, 15 * SW], F32, kind="ExternalOutput")
    out_pos = nc.dram_tensor("out_pos", [128, 2], F32, kind="ExternalOutput")

    with tile.TileContext(nc) as tc:
        import contextlib

        with contextlib.ExitStack() as ctx:
            const = ctx.enter_context(tc.tile_pool(name="const", bufs=1))
            big = ctx.enter_context(tc.tile_pool(name="big", bufs=1))
            stage = ctx.enter_context(tc.tile_pool(name="stage", bufs=5))
            sqp = ctx.enter_context(tc.tile_pool(name="sqp", bufs=4))
            lnpool = ctx.enter_context(tc.tile_pool(name="lnpool", bufs=2))
            bpool = ctx.enter_context(tc.tile_pool(name="bpool", bufs=3))
            esp = ctx.enter_context(tc.tile_pool(name="esp", bufs=8))
            sink = ctx.enter_context(tc.tile_pool(name="sink", bufs=2))

            ones_bf = const.tile([128, 128], BF16)
            nc.vector.memset(ones_bf[:], 1.0)
            ones_dr = const.tile([128, 2, 128], ZDT)
            nc.vector.memset(ones_dr[:], 1.0)
            ln_zs = const.tile([128, 1], F32)
            nc.vector.memset(ln_zs[:], float(math.log(ZSCALE)))

            # resident z tiles: per 1024-group, two chunk-pair tiles
            # [128, 2, GW] fp8 (pair A = K rows 0..255, pair B = 256..511)
            zq = [
                [big.tile([128, 2, GW], ZDT, tag=f"zq{g}{p}",
                          name=f"zq{g}{p}") for p in range(2)]
                for g in range(8)
            ]
            dacc = big.tile([128, 64], F32, tag="dacc")
            nc.vector.memset(dacc[:], 0.0)
            colrow = big.tile([1, 15 * SW], F32, tag="colrow")
            pos_acc = big.tile([128, 2], F32, tag="pos_acc")

            pp = ctx.enter_context(
                tc.tile_pool(name="pp", bufs=3, space="PSUM")
            )
            pc = ctx.enter_context(
                tc.tile_pool(name="pc", bufs=2, space="PSUM")
            )

            def prep(g):
                pt = pp.tile([128, GW], F32, tag="pp", name=f"pt{g}")
                sq = [sqp.tile([128, 2, GW], ZDT, tag="sqp",
                               name=f"sq{g}{p}") for p in range(2)]
                sts = []
                for p in range(2):
                    st = stage.tile([128, 2, GW], BF16, tag="stage",
                                    name=f"st{g}{p}")
                    for q in range(2):
                        nc.sync.dma_start(
                            st[:, q, :],
                            repsT[bass.ts(2 * p + q, 128), bass.ts(g, GW)],
                        )
                    sts.append(st)
                    eng = nc.gpsimd if (g * 2 + p) % 8 < SQ_GP8 else nc.vector
                    eng.tensor_mul(sq[p][:], st[:], st[:])
                if USE_DR:
                    for p in range(2):
                        for jj in range(2):
                            nc.tensor.matmul(
                                pt[:, bass.ts(jj, 512)],
                                ones_dr[:],
                                sq[p][:, :, bass.ts(jj, 512)],
                                start=(p == 0), stop=(p == 1),
                                perf_mode=DR,
                            )
                else:
                    for p in range(2):
                        for q in range(2):
                            for jj in range(2):
                                nc.tensor.matmul(
                                    pt[:, bass.ts(jj, 512)],
                                    ones_dr[:, 0, :],
                                    sq[p][:, q, bass.ts(jj, 512)],
                                    start=(p == 0 and q == 0),
                                    stop=(p == 1 and q == 1),
                                )
                lt = lnpool.tile([128, GW], BF16, tag="lnpool", name=f"lt{g}")
                nc.scalar.activation(lt[:], pt[:], AF.Ln)
                bt = bpool.tile([128, GW], BF16, tag="bpool", name=f"B{g}")
                nc.scalar.activation(bt[:], lt[:], AF.Exp, scale=-0.5,
                                     bias=ln_zs[:])
                for d in range(4):
                    nc.vector.tensor_mul(
                        zq[g][d // 2][:, d % 2, :], sts[d // 2][:, d % 2, :],
                        bt[:],
                    )

            es_tiles = {}

            def mains(si):
                s, g, off, w = SUBS[si]
                lg = 0 if s == 0 else 4  # lhsT group (local cstrip 0 / 8)
                for i in range(4):
                    pm = pp.tile([128, GW], F32, tag="pp",
                                 name=f"pm{si}_{i}")
                    if USE_DR:
                        for p in range(2):
                            for jj in range(w // 512):
                                nc.tensor.matmul(
                                    pm[:, bass.ts(jj, 512)],
                                    zq[lg][p][:, :, bass.ts(i, 128)],
                                    zq[g][p][:, :, off + jj * 512:
                                             off + jj * 512 + 512],
                                    start=(p == 0), stop=(p == 1),
                                    perf_mode=DR,
                                )
                    else:
                        for p in range(2):
                            for q in range(2):
                                for jj in range(w // 512):
                                    nc.tensor.matmul(
                                        pm[:, bass.ts(jj, 512)],
                                        zq[lg][p][:, q, bass.ts(i, 128)],
                                        zq[g][p][:, q, off + jj * 512:
                                                 off + jj * 512 + 512],
                                        start=(p == 0 and q == 0),
                                        stop=(p == 1 and q == 1),
                                    )
                    if i % 2 == 0:
                        es = esp.tile([128, 2, GW], ZDT, tag="esp",
                                      name=f"es{si}_{i}")
                        es_tiles[(si, i // 2)] = es
                    else:
                        es = es_tiles[(si, i // 2)]
                    slot = s * 32 + i * 8 + g
                    nc.scalar.activation(
                        es[:, i % 2, 0:w], pm[:, 0:w], AF.Exp,
                        scale=EXP_SCALE,
                        accum_out=dacc[:, slot:slot + 1],
                    )

            col_j = 0

            def colsums(si):
                nonlocal col_j
                if not USE_COLSUMS:
                    return
                s, g, off, w = SUBS[si]
                c0 = (g * GW + off) // SW
                for c in _sub_csubs(SUBS[si]):
                    eoff = (c - c0) * SW
                    pcT = pc.tile([128, SW], F32, tag="pc",
                                  name=f"pc{si}_{c}")
                    if USE_DR:
                        for ip in range(2):
                            nc.tensor.matmul(
                                pcT[:],
                                ones_dr[:],
                                es_tiles[(si, ip)][:, :, eoff:eoff + SW],
                                start=(ip == 0), stop=(ip == 1),
                                perf_mode=DR,
                            )
                    else:
                        for i in range(4):
                            nc.tensor.matmul(
                                pcT[:],
                                ones_bf[:],
                                es_tiles[(si, i // 2)][:, i % 2,
                                                       eoff:eoff + SW],
                                start=(i == 0), stop=(i == 3),
                            )
                    nc.vector.tensor_copy(
                        colrow[0:1, col_j * SW:(col_j + 1) * SW], pcT[0:1, :]
                    )
                    col_j += 1

            def pos_from_z():
                # sum_k cos_k = sum of all elements of (Z_i o Z_j) for the
                # positive pairs; those columns are local cols [0,512) of
                # groups 0 (rows) and 4 (rows+4096).  zq is z*16 so the
                # host divides the accumulated sum by 256.
                for p in range(2):
                    snk = sink.tile([128, 2, SW], BF16, tag="sink",
                                    name=f"snk{p}")
                    nc.vector.scalar_tensor_tensor(
                        snk[:], zq[0][p][:, :, 0:SW], 1.0,
                        zq[4][p][:, :, 0:SW],
                        ALU.mult, ALU.mult,
                        accum_out=pos_acc[:, p:p + 1],
                    )
                nc.sync.dma_start(out_pos[:], pos_acc[:])

            # ------- software-pipelined schedule -----------------------
            prep(0)
            prep(1)
            mains(0)
            prep(2)
            mains(1)
            colsums(0)
            prep(3)
            mains(2)
            colsums(1)
            prep(4)
            mains(3)
            colsums(2)
            prep(5)
            mains(4)
            colsums(3)
            if USE_POS:
                pos_from_z()
            else:
                nc.vector.memset(pos_acc[:], 0.0)
                nc.sync.dma_start(out_pos[:], pos_acc[:])
            prep(6)
            mains(5)
            colsums(4)
            prep(7)
            mains(6)
            colsums(5)
            mains(7)
            colsums(6)
            mains(8)
            colsums(7)
            colsums(8)
            nc.sync.dma_start(out_row[:], dacc[:])
            if USE_COLSUMS:
                nc.sync.dma_start(out_col[:], colrow[:])
            else:
                nc.vector.memset(colrow[:], 0.0)
                nc.sync.dma_start(out_col[:], colrow[:])

    nc.compile()
    return nc


_NC_CACHE = None


def _get_program():
    global _NC_CACHE
    if _NC_CACHE is None:
        _NC_CACHE = build_program()
    return _NC_CACHE


def make_in_maps(emb_i: np.ndarray, emb_j: np.ndarray):
    emb_i = np.asarray(emb_i, dtype=np.float32)
    emb_j = np.asarray(emb_j, dtype=np.float32)
    reps = np.concatenate([emb_i, emb_j], axis=0)          # [8192, 512]
    repsT = np.ascontiguousarray(reps.T).astype(ml_dtypes.bfloat16)
    in_maps = []
    for c in range(N_CORES):
        in_maps.append(
            {"repsT": np.ascontiguousarray(np.roll(repsT, -SW * c, axis=1))}
        )
    return in_maps


def combine_outputs(results):
    """Assemble denom from per-core partial row/col sums, then the loss."""
    rs = np.zeros(M, dtype=np.float64)
    cos_sum = 0.0
    for k, r in enumerate(results):
        dacc = np.asarray(r["out_row"], dtype=np.float64)    # [128, 64]
        for s, rstrip in enumerate((k, k + 8)):
            base = rstrip * SW
            for i in range(4):
                sl = dacc[:, s * 32 + i * 8:s * 32 + i * 8 + 8].sum(axis=1)
                rs[base + 128 * i: base + 128 * i + 128] += sl
        ocol = np.asarray(r["out_col"], dtype=np.float64).reshape(15, SW)
        for j, (si, c_loc) in enumerate(COLSUM_LIST):
            c_glob = (k + c_loc) % NSTRIP
            rs[c_glob * SW:(c_glob + 1) * SW] += ocol[j]
        cos_sum += float(np.asarray(r["out_pos"], dtype=np.float64).sum()) / (ZSCALE * ZSCALE)
    denom = rs - E2
    loss = (np.log(denom).sum() - 2.0 * INV_T * cos_sum) / float(M)
    return np.float32(loss)


def kernel(emb_i: np.ndarray, emb_j: np.ndarray) -> np.ndarray:
    nc = _get_program()
    in_maps = make_in_maps(emb_i, emb_j)
    res = run_bass_kernel_spmd(nc, in_maps, list(range(N_CORES)))
    return combine_outputs(res.results)


# revision 27
# speedup vs baseline: 1.1149x; 1.0638x over previous
"""NT-Xent contrastive loss on 8 Trainium2 NeuronCores (symmetric fp8 v3).

Math: z = l2-normalize rows of concat(emb_i, emb_j) -> [8192, 512].
sim = (z @ z.T)/T, T=0.5.  denom_r = sum_j exp(sim_rj) - e^2.
loss = (sum_r ln(denom_r) - 4*sum_k cos_k) / 8192.

sim is symmetric: only the upper triangle of the 16x16 grid of 512-row
strips is computed.  exp(sim) block (r,c) contributes its row sums to
denom[strip r] and its column sums to denom[strip c].  Round-robin
pairing makes the program uniform across cores: core k receives repsT
with columns rotated left by 512k, owns LOCAL row strips 0 and 8, and
computes strip 0 x local cstrips 0..8 plus strip 8 x local cstrips
8..15.  Over k=0..7 this covers each unordered strip pair exactly once.
Per-core partial row/col sums of exp go back to the host, which
assembles denom, takes float64 log, and forms the loss.

Device pipeline per core:
  - repsT arrives bf16 (host cast); per 1024-col group: DVE squares ->
    fp8, ones-weights DoubleRow matmul -> column sums-of-squares
    (PSUM, replicated over partitions), ACT ln then exp(-.5*ln+ln 16)
    -> B = 16/||col|| (bf16), DVE scale-mul -> z tiles (fp8, x16 to
    stay clear of fp8 denormals; exp scale compensates by 1/256)
  - mains: fp8 DoubleRow matmuls (K=512 as 2 passes of 2x128), PSUM
    [128,1024] groups, ACT exp(scale=2/256) -> es bf16 + accum_out row
    sums; colsum: ones-bf16 matmul chains over the 4 row tiles of each
    off-diag 512-block -> PSUM -> DMA one partition row to DRAM
  - positive pairs: separate bf16 row-major path (DVE fused
    multiply-reduce), cos_k per pair -> DRAM
  - ACT functions (Exp/Ln/Copy) pinned to one table set -> single
    ACT_TABLE_LOAD
"""

import functools
import math
import os

import numpy as np
import ml_dtypes

import concourse.bacc as bacc
import concourse.bass as bass
import concourse.tile as tile
from concourse import mybir
from concourse.bass_utils import run_bass_kernel_spmd
from concourse.hw_specs import get_activation_tables as _orig_gat

F32 = mybir.dt.float32
BF16 = mybir.dt.bfloat16
FP8 = mybir.dt.float8e4
AF = mybir.ActivationFunctionType
ALU = mybir.AluOpType
DR = mybir.MatmulPerfMode.DoubleRow

N_CORES = 8
N = 4096              # rows per input
D = 512               # embedding dim
M = 2 * N             # 8192 rows of sim
NSTRIP = 16           # 512-row strips
SW = 512              # strip width
GW = 1024             # column group width (PSUM group size)
POS_PER_CORE = N // N_CORES       # 512
E2 = float(math.exp(2.0))
INV_T = 2.0           # 1 / temperature
ZSCALE = 16.0         # fp8 z pre-scale (avoids fp8 denormals)
EXP_SCALE = INV_T / (ZSCALE * ZSCALE)

_ONE_SET = "natural_log_exp_and_others"

# mains subgroups, uniform for every core (local indices):
#   (strip_sel, group, col_off, width); strip A = local strip 0
#   (lhsT = group 0 cols [0,512)), strip B = local strip 8 (lhsT =
#   group 4 cols [0,512)).
SUBS = (
    (0, 0, 0, 1024),
    (0, 1, 0, 1024),
    (0, 2, 0, 1024),
    (0, 3, 0, 1024),
    (0, 4, 0, 512),
    (1, 4, 0, 1024),
    (1, 5, 0, 1024),
    (1, 6, 0, 1024),
    (1, 7, 0, 1024),
)
_DIAG_CSUB = {0: 0, 1: 8}  # strip_sel -> local diag cstrip


def _sub_csubs(sub):
    """Local 512-col strips covered by a mains subgroup, with the
    diagonal one excluded (those need no colsum)."""
    s, g, off, w = sub
    c0 = (g * GW + off) // SW
    return [c for c in range(c0, c0 + w // SW) if c != _DIAG_CSUB[s]]


COLSUM_LIST = [(si, c) for si, sub in enumerate(SUBS) for c in _sub_csubs(sub)]
assert len(COLSUM_LIST) == 15
# slot offset of each sub's first colsum vector in out_col (si-ordered)
COL_OFF = {}
for _j, (_si, _c) in enumerate(COLSUM_LIST):
    COL_OFF.setdefault(_si, _j)


@functools.cache
def _patched_gat(arch):
    """Pin every ACT function this kernel uses to one table set so the
    table-load chooser emits a single ACT_TABLE_LOAD."""
    t = dict(_orig_gat(arch))
    if _ONE_SET not in t:
        return t
    mine = {AF.Exp, AF.Ln, AF.Square, AF.Copy, AF.Identity}
    return {
        name: (s if name == _ONE_SET else (set(s) - mine))
        for name, s in t.items()
    }


USE_BF16 = os.environ.get("K_BF16", "") != ""         # bf16 instead of fp8
USE_DR = os.environ.get("K_NO_DR", "") == "" and not USE_BF16
USE_COLSUMS = os.environ.get("K_NO_COLSUMS", "") == ""
USE_POS = os.environ.get("K_NO_POS", "") == ""
ZDT = BF16 if USE_BF16 else FP8
SQ_GP8 = int(os.environ.get("K_SQ_GP8", "5"))   # of every 8 squares, this many on gpsimd
POS_GP = os.environ.get("K_POS_DVE", "") == ""  # pos products on gpsimd


def build_program():
    bacc.get_activation_tables = _patched_gat

    nc = bacc.Bacc(
        "TRN2",
        target_bir_lowering=False,
        debug=False,
        num_devices=N_CORES,
    )

    repsT = nc.dram_tensor("repsT", [D, M], BF16, kind="ExternalInput")
    out_row = nc.dram_tensor("out_row", [128, 64], F32, kind="ExternalOutput")
    out_col = nc.dram_tensor("out_col", [1, 15 * SW], F32, kind="ExternalOutput")
    out_pos = nc.dram_tensor("out_pos", [128, 2], F32, kind="ExternalOutput")

    with tile.TileContext(nc) as tc:
        import contextlib

        with contextlib.ExitStack() as ctx:
            const = ctx.enter_context(tc.tile_pool(name="const", bufs=1))
            big = ctx.enter_context(tc.tile_pool(name="big", bufs=1))
            stage = ctx.enter_context(tc.tile_pool(name="stage", bufs=5))
            sqp = ctx.enter_context(tc.tile_pool(name="sqp", bufs=4))
            lnpool = ctx.enter_context(tc.tile_pool(name="lnpool", bufs=2))
            bpool = ctx.enter_context(tc.tile_pool(name="bpool", bufs=3))
            esp = ctx.enter_context(tc.tile_pool(name="esp", bufs=8))
            sink = ctx.enter_context(tc.tile_pool(name="sink", bufs=2))

            ones_bf = const.tile([128, 128], BF16)
            nc.vector.memset(ones_bf[:], 1.0)
            ones_dr = const.tile([128, 2, 128], ZDT)
            nc.vector.memset(ones_dr[:], 1.0)
            ln_zs = const.tile([128, 1], F32)
            nc.vector.memset(ln_zs[:], float(math.log(ZSCALE)))

            # resident z tiles: per 1024-group, two chunk-pair tiles
            # [128, 2, GW] fp8 (pair A = K rows 0..255, pair B = 256..511)
            zq = [
                [big.tile([128, 2, GW], ZDT, tag=f"zq{g}{p}",
                          name=f"zq{g}{p}") for p in range(2)]
                for g in range(8)
            ]
            dacc = big.tile([128, 64], F32, tag="dacc")
            nc.vector.memset(dacc[:], 0.0)
            colrow = big.tile([1, 15 * SW], F32, tag="colrow")
            pos_acc = big.tile([128, 2], F32, tag="pos_acc")

            pp = ctx.enter_context(
                tc.tile_pool(name="pp", bufs=3, space="PSUM")
            )
            pc = ctx.enter_context(
                tc.tile_pool(name="pc", bufs=2, space="PSUM")
            )

            def prep(g):
                pt = pp.tile([128, GW], F32, tag="pp", name=f"pt{g}")
                sq = [sqp.tile([128, 2, GW], ZDT, tag="sqp",
                               name=f"sq{g}{p}") for p in range(2)]
                sts = []
                for p in range(2):
                    st = stage.tile([128, 2, GW], BF16, tag="stage",
                                    name=f"st{g}{p}")
                    for q in range(2):
                        nc.sync.dma_start(
                            st[:, q, :],
                            repsT[bass.ts(2 * p + q, 128), bass.ts(g, GW)],
                        )
                    sts.append(st)
                    if g in (0, 4):
                        # lhsT groups gate the mains: use the (startup-idle)
                        # ACT engine so neither DVE nor slow gpsimd delays them
                        nc.scalar.activation(sq[p][:], st[:], AF.Square)
                    elif (g * 2 + p) % 3 != 2:
                        nc.gpsimd.tensor_mul(sq[p][:], st[:], st[:])
                    else:
                        nc.vector.tensor_mul(sq[p][:], st[:], st[:])
                if USE_DR:
                    for p in range(2):
                        for jj in range(2):
                            nc.tensor.matmul(
                                pt[:, bass.ts(jj, 512)],
                                ones_dr[:],
                                sq[p][:, :, bass.ts(jj, 512)],
                                start=(p == 0), stop=(p == 1),
                                perf_mode=DR,
                            )
                else:
                    for p in range(2):
                        for q in range(2):
                            for jj in range(2):
                                nc.tensor.matmul(
                                    pt[:, bass.ts(jj, 512)],
                                    ones_dr[:, 0, :],
                                    sq[p][:, q, bass.ts(jj, 512)],
                                    start=(p == 0 and q == 0),
                                    stop=(p == 1 and q == 1),
                                )
                lt = lnpool.tile([128, GW], BF16, tag="lnpool", name=f"lt{g}")
                nc.scalar.activation(lt[:], pt[:], AF.Ln)
                bt = bpool.tile([128, GW], BF16, tag="bpool", name=f"B{g}")
                nc.scalar.activation(bt[:], lt[:], AF.Exp, scale=-0.5,
                                     bias=ln_zs[:])
                for d in range(4):
                    nc.vector.tensor_mul(
                        zq[g][d // 2][:, d % 2, :], sts[d // 2][:, d % 2, :],
                        bt[:],
                    )

            es_tiles = {}

            def mains(si):
                s, g, off, w = SUBS[si]
                lg = 0 if s == 0 else 4  # lhsT group (local cstrip 0 / 8)
                for i in range(4):
                    pm = pp.tile([128, GW], F32, tag="pp",
                                 name=f"pm{si}_{i}")
                    if USE_DR:
                        for p in range(2):
                            for jj in range(w // 512):
                                nc.tensor.matmul(
                                    pm[:, bass.ts(jj, 512)],
                                    zq[lg][p][:, :, bass.ts(i, 128)],
                                    zq[g][p][:, :, off + jj * 512:
                                             off + jj * 512 + 512],
                                    start=(p == 0), stop=(p == 1),
                                    perf_mode=DR,
                                )
                    else:
                        for p in range(2):
                            for q in range(2):
                                for jj in range(w // 512):
                                    nc.tensor.matmul(
                                        pm[:, bass.ts(jj, 512)],
                                        zq[lg][p][:, q, bass.ts(i, 128)],
                                        zq[g][p][:, q, off + jj * 512:
                                                 off + jj * 512 + 512],
                                        start=(p == 0 and q == 0),
                                        stop=(p == 1 and q == 1),
                                    )
                    if i % 2 == 0:
                        es = esp.tile([128, 2, GW], ZDT, tag="esp",
                                      name=f"es{si}_{i}")
                        es_tiles[(si, i // 2)] = es
                    else:
                        es = es_tiles[(si, i // 2)]
                    slot = s * 32 + i * 8 + g
                    nc.scalar.activation(
                        es[:, i % 2, 0:w], pm[:, 0:w], AF.Exp,
                        scale=EXP_SCALE,
                        accum_out=dacc[:, slot:slot + 1],
                    )

            def colsums(si):
                if not USE_COLSUMS:
                    return
                s, g, off, w = SUBS[si]
                c0 = (g * GW + off) // SW
                col_j = COL_OFF[si]
                for c in _sub_csubs(SUBS[si]):
                    eoff = (c - c0) * SW
                    pcT = pc.tile([128, SW], F32, tag="pc",
                                  name=f"pc{si}_{c}")
                    if USE_DR:
                        for ip in range(2):
                            nc.tensor.matmul(
                                pcT[:],
                                ones_dr[:],
                                es_tiles[(si, ip)][:, :, eoff:eoff + SW],
                                start=(ip == 0), stop=(ip == 1),
                                perf_mode=DR,
                            )
                    else:
                        for i in range(4):
                            nc.tensor.matmul(
                                pcT[:],
                                ones_bf[:],
                                es_tiles[(si, i // 2)][:, i % 2,
                                                       eoff:eoff + SW],
                                start=(i == 0), stop=(i == 3),
                            )
                    nc.vector.tensor_copy(
                        colrow[0:1, col_j * SW:(col_j + 1) * SW], pcT[0:1, :]
                    )
                    col_j += 1

            def pos_from_z():
                # sum_k cos_k = sum of all elements of (Z_i o Z_j) for the
                # positive pairs; those columns are local cols [0,512) of
                # groups 0 (rows) and 4 (rows+4096).  zq is z*16 so the
                # host divides the accumulated sum by 256.
                for p in range(2):
                    snk = sink.tile([128, 2, SW], BF16, tag="sink",
                                    name=f"snk{p}")
                    nc.vector.scalar_tensor_tensor(
                        snk[:], zq[0][p][:, :, 0:SW], 1.0,
                        zq[4][p][:, :, 0:SW],
                        ALU.mult, ALU.mult,
                        accum_out=pos_acc[:, p:p + 1],
                    )
                nc.sync.dma_start(out_pos[:], pos_acc[:])

            # ------- software-pipelined schedule -----------------------
            # prep lhsT groups (0 for strip A, 4 for strip B) first so two
            # independent mains streams open up as early as possible
            prep(0)
            prep(4)
            mains(0)
            prep(1)
            mains(5)
            if USE_POS:
                pos_from_z()
            prep(5)
            mains(1)
            colsums(0)
            prep(2)
            mains(6)
            colsums(5)
            prep(6)
            mains(2)
            colsums(1)
            prep(3)
            mains(7)
            colsums(6)
            prep(7)
            mains(3)
            colsums(2)
            mains(8)
            colsums(7)
            mains(4)
            colsums(3)
            colsums(8)
            colsums(4)
            if not USE_POS:
                nc.vector.memset(pos_acc[:], 0.0)
                nc.sync.dma_start(out_pos[:], pos_acc[:])
            nc.sync.dma_start(out_row[:], dacc[:])
            if USE_COLSUMS:
                nc.sync.dma_start(out_col[:], colrow[:])
            else:
                nc.vector.memset(colrow[:], 0.0)
                nc.sync.dma_start(out_col[:], colrow[:])

    nc.compile()
    return nc


_NC_CACHE = None


def _get_program():
    global _NC_CACHE
    if _NC_CACHE is None:
        _NC_CACHE = build_program()
    return _NC_CACHE


def make_in_maps(emb_i: np.ndarray, emb_j: np.ndarray):
    emb_i = np.asarray(emb_i, dtype=np.float32)
    emb_j = np.asarray(emb_j, dtype=np.float32)
    reps = np.concatenate([emb_i, emb_j], axis=0)          # [8192, 512]
    repsT = np.ascontiguousarray(reps.T).astype(ml_dtypes.bfloat16)
    in_maps = []
    for c in range(N_CORES):
        in_maps.append(
            {"repsT": np.ascontiguousarray(np.roll(repsT, -SW * c, axis=1))}
        )
    return in_maps


def combine_outputs(results):
    """Assemble denom from per-core partial row/col sums, then the loss."""
    rs = np.zeros(M, dtype=np.float64)
    cos_sum = 0.0
    for k, r in enumerate(results):
        dacc = np.asarray(r["out_row"], dtype=np.float64)    # [128, 64]
        for s, rstrip in enumerate((k, k + 8)):
            base = rstrip * SW
            for i in range(4):
                sl = dacc[:, s * 32 + i * 8:s * 32 + i * 8 + 8].sum(axis=1)
                rs[base + 128 * i: base + 128 * i + 128] += sl
        ocol = np.asarray(r["out_col"], dtype=np.float64).reshape(15, SW)
        for j, (si, c_loc) in enumerate(COLSUM_LIST):
            c_glob = (k + c_loc) % NSTRIP
            rs[c_glob * SW:(c_glob + 1) * SW] += ocol[j]
        cos_sum += float(np.asarray(r["out_pos"], dtype=np.float64).sum()) / (ZSCALE * ZSCALE)
    denom = rs - E2
    loss = (np.log(denom).sum() - 2.0 * INV_T * cos_sum) / float(M)
    return np.float32(loss)


def kernel(emb_i: np.ndarray, emb_j: np.ndarray) -> np.ndarray:
    nc = _get_program()
    in_maps = make_in_maps(emb_i, emb_j)
    res = run_bass_kernel_spmd(nc, in_maps, list(range(N_CORES)))
    return combine_outputs(res.results)
